# revision 35
# baseline (speedup 1.0000x reference)
"""Trainium2 Bass kernel for nn_Attention_3736621547687.

B=1, S=2048, HID=2048, NH=16, NKV=4, HD=128 attention block:
qkv proj -> per-head RMSNorm(q,k) -> RoPE -> causal GQA attention -> o proj.

Sharding: tensor-parallel over heads across 8 cores. Core c owns q heads
{2c, 2c+1} and kv head c//2 (replicated across the pair of cores sharing it).
Each core computes a partial o-projection output; the host sums the 8
partials (Megatron-style row-parallel reduce) and adds the output bias.

Device-side layout trick: everything is computed in "transposed" orientation
(feature dim on partitions, sequence on the free dim) so that no on-chip
transposes of activations are needed:
  - host supplies hidden^T, wqkv_c^T, wo_c^T, cos/sin tiled to [128, S],
    all pre-converted to bf16 so HBM traffic is halved
  - qkv proj emits q^T/k^T/v^T directly
  - scores are computed as scores^T [keys, queries]; softmax denominators are
    partition-dim sums obtained with an all-ones [128,128] matmul that also
    replicates the result across partitions (giving the broadcast for free)
  - exp() is fused with the 1/sqrt(qpa) scale on the scalar engine; causal
    masking = zeroing exp values above the diagonal with affine_select
    (identical to the reference's additive -1e9 mask in fp32)
  - softmax max-subtraction is skipped: scores are ~N(0,1) after RMSNorm so
    exp() cannot overflow; mathematically identical to the reference.
All matmuls run in bf16 (full PE rate); PSUM accumulation is fp32.
A burst of identity warm-up matmuls at kernel start flips the PE HAM clock
gate to 2.4 GHz while the first weight/activation DMAs are still in flight.
"""

import numpy as np
from contextlib import ExitStack

import concourse.bass as bass
import concourse.bacc as bacc
import concourse.mybir as mybir
import concourse.tile as tile
from concourse.masks import make_identity
from concourse.bass_utils import run_bass_kernel_spmd

S = 2048
HID = 2048
NH = 16
NKV = 4
HD = 128
G = NH // NKV
SCALE = float(128.0 ** -0.5)  # query_pre_attn_scalar = 128
EPS = 1e-6

FP32 = mybir.dt.float32
F32R = mybir.dt.float32r
BF16 = mybir.dt.bfloat16
MULT = mybir.AluOpType.mult
AF = mybir.ActivationFunctionType

N_CORES = 8
SC_ATT = 512    # moving-dim chunk for every stage


def _patch_act_tables():
    """Force Ln and Exp onto the single combined activation-table set so the
    scalar engine never reloads tables when rms-norm and softmax interleave.
    Set ids must keep their positions, so competing sets are emptied rather
    than removed."""
    import concourse.hw_specs as hw_specs
    import concourse.bacc as bacc_mod
    orig = hw_specs.get_activation_tables

    def patched(module_arch):
        t = orig(module_arch)
        for name in ("exp_and_others", "natural_log", "exp_and_friends"):
            if name in t and "natural_log_exp_and_others" in t:
                t[name] = set()
        return t

    bacc_mod.get_activation_tables = patched


def build_nc():
    _patch_act_tables()
    nc = bacc.Bacc()

    hT = nc.dram_tensor("hT", [128, 16, S], BF16, kind="ExternalInput")
    wts_d = [nc.dram_tensor(f"wT{i}", [128, 16, HD], BF16, kind="ExternalInput")
             for i in range(4)]
    b4 = nc.dram_tensor("b4", [128, 4], FP32, kind="ExternalInput")
    woT = nc.dram_tensor("woT", [128, 2, HID], BF16, kind="ExternalInput")
    cs2 = nc.dram_tensor("cs2", [128, S], BF16, kind="ExternalInput")
    ss2 = nc.dram_tensor("ss2", [128, S], BF16, kind="ExternalInput")
    qw = nc.dram_tensor("qw", [128, 1], FP32, kind="ExternalInput")
    qws = nc.dram_tensor("qws", [128, 1], FP32, kind="ExternalInput")
    kw = nc.dram_tensor("kw", [128, 1], FP32, kind="ExternalInput")
    kws = nc.dram_tensor("kws", [128, 1], FP32, kind="ExternalInput")
    onesd = nc.dram_tensor("ones", [128, 128], F32R, kind="ExternalInput")
    onesbd = nc.dram_tensor("onesb", [128, 128], BF16, kind="ExternalInput")
    rswapd = nc.dram_tensor("rswap", [128, 128], BF16, kind="ExternalInput")
    tmaskd = nc.dram_tensor("tmask", [128, 2 * SC_ATT], BF16,
                            kind="ExternalInput")
    outp = nc.dram_tensor("outp", [S, HID], BF16, kind="ExternalOutput")

    with ExitStack() as ctx:
        tc = ctx.enter_context(tile.TileContext(nc))

        const = ctx.enter_context(tc.tile_pool(name="const", bufs=1))
        hpool = ctx.enter_context(tc.tile_pool(name="hpool", bufs=2))
        rawp = ctx.enter_context(tc.tile_pool(name="rawp", bufs=1))
        atp = ctx.enter_context(tc.tile_pool(name="atp", bufs=1))
        wpool = ctx.enter_context(tc.tile_pool(name="wpool", bufs=3))
        vpool = ctx.enter_context(tc.tile_pool(name="vpool", bufs=1))
        ppool = ctx.enter_context(tc.tile_pool(name="ppool", bufs=2))
        dpool = ctx.enter_context(tc.tile_pool(name="dpool", bufs=2))
        opool = ctx.enter_context(tc.tile_pool(name="opool", bufs=2))

        psA = ctx.enter_context(tc.tile_pool(name="psA", bufs=1, space="PSUM"))
        psB = ctx.enter_context(tc.tile_pool(name="psB", bufs=2, space="PSUM"))
        psC = ctx.enter_context(tc.tile_pool(name="psC", bufs=3, space="PSUM"))
        psO = ctx.enter_context(tc.tile_pool(name="psO", bufs=2, space="PSUM"))

        # ---- identity first: it feeds the PE warm-up matmuls ----------
        ident = const.tile([128, 128], BF16)
        make_identity(nc, ident)

        # ---- weight loads on the sync HWDGE queue (k-head slice first) -
        b4s = const.tile([128, 4], FP32)
        nc.sync.dma_start(out=b4s, in_=b4[:, :])
        wts = [None] * 4
        for oc in (2, 0, 1, 3):
            wt = const.tile([128, 16, HD], BF16, name=f"wts{oc}",
                            tag=f"wts{oc}")
            # split halves so the first proj group can start on kt 0-7
            # while the rest of the weights stream in
            nc.sync.dma_start(out=wt[:, 0:8, :], in_=wts_d[oc][:, 0:8, :])
            nc.sync.dma_start(out=wt[:, 8:16, :], in_=wts_d[oc][:, 8:16, :])
            wts[oc] = wt

        # ---- small constants on the SWDGE queue ------------------------
        ones128 = const.tile([128, 128], F32R)
        nc.gpsimd.dma_start(out=ones128, in_=onesd[:, :])
        onesb = const.tile([128, 128], BF16)
        nc.gpsimd.dma_start(out=onesb, in_=onesbd[:, :])
        Rm = const.tile([128, 128], BF16)
        nc.gpsimd.dma_start(out=Rm, in_=rswapd[:, :])
        tmask = const.tile([128, 2 * SC_ATT], BF16)
        nc.gpsimd.dma_start(out=tmask, in_=tmaskd[:, :])
        qw_ = const.tile([128, 1], FP32)
        nc.gpsimd.dma_start(out=qw_, in_=qw[:, :])
        qws_ = const.tile([128, 1], FP32)
        nc.gpsimd.dma_start(out=qws_, in_=qws[:, :])
        kw_ = const.tile([128, 1], FP32)
        nc.gpsimd.dma_start(out=kw_, in_=kw[:, :])
        kws_ = const.tile([128, 1], FP32)
        nc.gpsimd.dma_start(out=kws_, in_=kws[:, :])
        epsc = const.tile([128, 1], FP32)
        nc.vector.memset(epsc, EPS)

        # ---- PE warm-up: ~4us of dummy matmuls so the HAM clock gate is
        # at 2.4 GHz by the time the first weight DMA lands ---------------
        warm = psC.tile([128, SC_ATT], FP32, tag="score")
        for _ in range(40):
            nc.tensor.matmul(warm[:, 0:128], lhsT=ident, rhs=ident,
                             start=True, stop=True)

        rawq = [rawp.tile([128, S], BF16, tag=f"raw{i}", name=f"raw{i}")
                for i in range(3)]
        qhat = [atp.tile([128, S], BF16, tag=f"qh{i}", name=f"qh{i}")
                for i in range(2)]
        khat = atp.tile([128, S], BF16, tag="kh")
        attnT = [atp.tile([128, S], BF16, tag=f"attnT{h}", name=f"attnT{h}")
                 for h in range(2)]
        vsb = vpool.tile([128, 16, HD], BF16, tag="vsb")
        heads = [
            (rawq[2], khat, kw_, kws_),
            (rawq[0], qhat[0], qw_, qws_),
            (rawq[1], qhat[1], qw_, qws_),
        ]
        raw3s = {}
        pending_norm = []

        def emit_norm(item):
            outps_, dacc_, h_, sl_ = item
            drep = psC.tile([128, SC_ATT], FP32, tag="score")
            nc.tensor.matmul(drep, lhsT=ones128, rhs=dacc_,
                             start=True, stop=True)
            drec = wpool.tile([128, SC_ATT], FP32, tag="tt", bufs=3)
            nc.vector.reciprocal_approx_fast(drec, drep)
            nc.vector.tensor_mul(attnT[h_][:, sl_], outps_, drec)

        # ================================================================
        # Software-pipelined emission: stage lag guarantees every
        # instruction's inputs are a full pipeline iteration old, so no
        # engine stream ever blocks at a phase boundary.
        #   iter sc: proj(sc) | rope+V(sc-1) | attention(sc-2) | oproj(sc-3)
        # ================================================================
        NCH = S // SC_ATT
        for it in range(NCH + 3):
            # ---- stage 1: qkv projection ------------------------------
            if it < NCH:
                sc = it
                sl = bass.ts(sc, SC_ATT)
                htsA = hpool.tile([128, 8, SC_ATT], BF16, tag="htsA")
                htsB = hpool.tile([128, 8, SC_ATT], BF16, tag="htsB")
                if sc == 0:
                    # fine-grained first load: earliest possible proj start
                    for q in range(4):
                        dst = htsA if q < 2 else htsB
                        nc.scalar.dma_start(
                            out=dst[:, 4 * (q % 2):4 * (q % 2) + 4, :],
                            in_=hT[:, 4 * q:4 * q + 4, sl])
                else:
                    nc.scalar.dma_start(out=htsA, in_=hT[:, 0:8, sl])
                    nc.scalar.dma_start(out=htsB, in_=hT[:, 8:16, sl])
                if sc == 1:
                    # deferred constants ride the scalar queue behind the
                    # chunk-1 activations: cos/sin first needed by rope(0)
                    # at ~25us, o-proj weights at iteration 3
                    cs2s = const.tile([128, S], BF16)
                    nc.scalar.dma_start(out=cs2s, in_=cs2[:, :])
                    ss2s = const.tile([128, S], BF16)
                    nc.scalar.dma_start(out=ss2s, in_=ss2[:, :])
                    woTs = const.tile([128, 2, HID], BF16)
                    nc.scalar.dma_start(out=woTs, in_=woT[:, :, :])
                for oc in (2, 0, 1, 3):
                    ps = psA.tile([128, SC_ATT], FP32, tag="mm")
                    for kt in range(16):
                        src_h = htsA if kt < 8 else htsB
                        nc.tensor.matmul(
                            ps, lhsT=wts[oc][:, kt, :],
                            rhs=src_h[:, kt % 8, :],
                            start=(kt == 0), stop=(kt == 15))
                    if oc == 3:
                        raw3 = wpool.tile([128, SC_ATT], BF16, tag="raw3",
                                          bufs=2)
                        nc.vector.tensor_scalar_add(raw3, ps,
                                                    b4s[:, oc:oc + 1])
                        raw3s[sc] = raw3
                    else:
                        nc.vector.tensor_scalar_add(
                            rawq[oc][:, sl], ps, b4s[:, oc:oc + 1])

            # ---- stage 2: rmsnorm + rope + V tiles --------------------
            if 1 <= it <= NCH:
                sc = it - 1
                sl = bass.ts(sc, SC_ATT)
                for raw, dst, w_, wsw_ in heads:
                    sq = wpool.tile([128, SC_ATT], BF16, tag="sq", bufs=2)
                    nc.vector.tensor_mul(sq, raw[:, sl], raw[:, sl])
                    ssum = psC.tile([128, SC_ATT], FP32, tag="score")
                    nc.tensor.matmul(ssum, lhsT=onesb, rhs=sq,
                                     start=True, stop=True)
                    lnb = wpool.tile([128, SC_ATT], FP32, tag="lnb", bufs=2)
                    nc.scalar.activation(lnb, ssum, AF.Ln,
                                         scale=1.0 / HD, bias=epsc)
                    nc.scalar.activation(lnb, lnb, AF.Exp, scale=-0.5)
                    rtp = psC.tile([128, SC_ATT], FP32, tag="score")
                    nc.tensor.matmul(rtp, lhsT=Rm, rhs=raw[:, sl],
                                     start=True, stop=True)
                    t1 = wpool.tile([128, SC_ATT], BF16, tag="tt", bufs=3)
                    nc.vector.scalar_tensor_tensor(
                        t1, in0=raw[:, sl], scalar=w_, in1=cs2s[:, sl],
                        op0=MULT, op1=MULT)
                    t2 = wpool.tile([128, SC_ATT], BF16, tag="tt", bufs=3)
                    nc.vector.scalar_tensor_tensor(
                        t2, in0=rtp, scalar=wsw_, in1=ss2s[:, sl],
                        op0=MULT, op1=MULT)
                    t3 = wpool.tile([128, SC_ATT], BF16, tag="tt", bufs=3)
                    nc.vector.tensor_add(t3, t1, t2)
                    nc.vector.tensor_mul(dst[:, sl], t3, lnb)
                raw3 = raw3s.pop(sc)
                for j in range(4):
                    tt = 4 * sc + j
                    vps = psC.tile([128, SC_ATT], BF16, tag="score")
                    nc.tensor.transpose(vps[:, 0:128],
                                        raw3[:, bass.ts(j, 128)], ident)
                    nc.vector.tensor_copy(vsb[:, tt, :], vps[:, 0:128])

            # ---- stage 3: attention, both heads -----------------------
            if 2 <= it <= NCH + 1:
                sc = it - 2
                sl = bass.ts(sc, SC_ATT)
                s0 = sc * SC_ATT
                ntt = sc * 4 + 4
                for h in range(2):
                    # softmax normalization of the PREVIOUS head is emitted
                    # here so its ones-matmul (which waits on the vector
                    # dacc chain) sits behind this head's score matmuls in
                    # the tensor FIFO instead of stalling it
                    if pending_norm:
                        emit_norm(pending_norm.pop(0))
                    outps = psO.tile([128, SC_ATT], FP32, tag="attnout")
                    dacc = dpool.tile([128, SC_ATT], F32R, tag="dacc")
                    for g in range(ntt // 4):
                        pg = ppool.tile([128, 4, SC_ATT], BF16, tag="pt",
                                        bufs=3)
                        for j in range(4):
                            tt = g * 4 + j
                            band = tt >= sc * 4  # diagonal band: mask t > s
                            scp = psC.tile([128, SC_ATT], FP32, tag="score")
                            nc.tensor.matmul(
                                scp, lhsT=khat[:, bass.ts(tt, 128)],
                                rhs=qhat[h][:, sl],
                                start=True, stop=not band)
                            if band:
                                # additive causal mask: -1e9 where q < k,
                                # identical to the reference's mask add.
                                # tmask[p, SC+c] = -1e9*[c < p]; column c
                                # of this tile is global query s0+c vs key
                                # 128*tt+p  ->  c_local = c - 128*(tt-4*sc)
                                off = SC_ATT - 128 * (tt - 4 * sc)
                                nc.tensor.matmul(
                                    scp, lhsT=ident,
                                    rhs=tmask[:, bass.ds(off, SC_ATT)],
                                    start=False, stop=True)
                            nc.scalar.activation(pg[:, j, :], scp, AF.Exp,
                                                 scale=SCALE)
                            nc.tensor.matmul(outps, lhsT=vsb[:, tt, :],
                                             rhs=pg[:, j, :],
                                             start=(tt == 0),
                                             stop=(tt == ntt - 1))
                        # group reduction split across vector + gpsimd
                        ga = ppool.tile([128, SC_ATT], BF16, tag="ga",
                                        bufs=2)
                        nc.vector.tensor_add(ga, pg[:, 0, :], pg[:, 1, :])
                        gb = ppool.tile([128, SC_ATT], BF16, tag="gb",
                                        bufs=2)
                        nc.gpsimd.tensor_add(gb, pg[:, 2, :], pg[:, 3, :])
                        if g == 0:
                            nc.vector.tensor_add(dacc, ga, gb)
                        else:
                            gc = ppool.tile([128, SC_ATT], BF16, tag="ga",
                                            bufs=2)
                            nc.vector.tensor_add(gc, ga, gb)
                            nc.vector.tensor_add(dacc, dacc, gc)
                    pending_norm.append((outps, dacc, h, sl))

            # ---- stage 4: o projection --------------------------------
            if it >= 3:
                sc = it - 3
                if sc == NCH - 1:
                    while pending_norm:
                        emit_norm(pending_norm.pop(0))
                ti = 0
                for st in range(4 * sc, 4 * sc + 4):
                    for jc in range(HID // SC_ATT):
                        if sc == NCH - 1:
                            # drain: proj + score PSUM banks are free now;
                            # rotate through them for 5-deep buffering
                            pool, tg = [(psB, "omm"), (psA, "mm"),
                                        (psC, "score")][ti % 3]
                            ops = pool.tile([128, SC_ATT], FP32, tag=tg)
                        elif sc >= 1:
                            # qkv proj is done by iteration 4: its PSUM
                            # bank joins the o-proj rotation (3 slots)
                            pool, tg = [(psB, "omm"), (psB, "omm"),
                                        (psA, "mm")][ti % 3]
                            ops = pool.tile([128, SC_ATT], FP32, tag=tg)
                        else:
                            ops = psB.tile([128, SC_ATT], FP32, tag="omm")
                        ti += 1
                        for h in range(2):
                            nc.tensor.matmul(
                                ops, lhsT=attnT[h][:, bass.ts(st, 128)],
                                rhs=woTs[:, h, bass.ts(jc, SC_ATT)],
                                start=(h == 0), stop=(h == 1))
                        osb = opool.tile([128, SC_ATT], BF16, tag="osb",
                                         bufs=4)
                        # halve the PSUM-bank hold time: both engines copy
                        # one half of the tile in parallel
                        nc.vector.tensor_copy(osb[:, 0:256], ops[:, 0:256])
                        nc.scalar.copy(osb[:, 256:512], ops[:, 256:512])
                        nc.sync.dma_start(
                            out=outp[bass.ts(st, 128), bass.ts(jc, SC_ATT)],
                            in_=osb)

    nc.compile()
    return nc


def _prep_inputs(hidden_states, cos, sin, wqkv, bqkv, wo, q_norm_w, k_norm_w):
    """Host-side layout prep + per-core sharding. All device tensors are
    pre-swizzled (and converted to bf16) so every DMA has long contiguous
    per-partition runs at half the fp32 byte count."""
    import ml_dtypes
    f32 = np.float32
    bf16 = ml_dtypes.bfloat16
    hTn = np.ascontiguousarray(hidden_states.reshape(S, HID).T).astype(f32)
    hTh = np.ascontiguousarray(
        hTn.reshape(16, 128, S).transpose(1, 0, 2)).astype(bf16)  # [p, kt, s]
    cosT = cos.T.astype(f32)  # [64, S]
    sinT = sin.T.astype(f32)
    cs2 = np.ascontiguousarray(
        np.concatenate([cosT, cosT], axis=0)).astype(bf16)
    ss2 = np.ascontiguousarray(
        np.concatenate([sinT, sinT], axis=0)).astype(bf16)
    qw = np.ascontiguousarray(q_norm_w.reshape(128, 1)).astype(f32)
    qws = np.ascontiguousarray(
        np.concatenate([q_norm_w[64:], q_norm_w[:64]]).reshape(128, 1)).astype(f32)
    kw = np.ascontiguousarray(k_norm_w.reshape(128, 1)).astype(f32)
    kws = np.ascontiguousarray(
        np.concatenate([k_norm_w[64:], k_norm_w[:64]]).reshape(128, 1)).astype(f32)
    ones_np = np.ones((128, 128), dtype=f32)
    onesb_np = np.ones((128, 128), dtype=bf16)
    rt = np.zeros((128, 128), dtype=f32)
    rt[np.arange(64) + 64, np.arange(64)] = -1.0   # R^T[d+64, d] = -1
    rt[np.arange(64), np.arange(64) + 64] = 1.0    # R^T[d-64, d] = +1
    rt = rt.astype(bf16)
    # causal mask template: tmask[p, 512+d] = -1e9 where d < p
    dcol = np.arange(2 * 512)[None, :] - 512
    prow = np.arange(128)[:, None]
    tmask = np.where(dcol < prow, np.float32(-1e9), np.float32(0.0)).astype(bf16)

    in_maps = []
    for c in range(N_CORES):
        kvh = c // 2
        rows = list(range(2 * c * HD, (2 * c + 2) * HD))          # q0, q1
        rows += list(range(NH * HD + kvh * HD, NH * HD + (kvh + 1) * HD))  # k
        rows += list(range((NH + NKV) * HD + kvh * HD,
                           (NH + NKV) * HD + (kvh + 1) * HD))      # v
        w_c = wqkv[rows]                       # [512, HID]
        wTc = np.ascontiguousarray(w_c.T).astype(f32)   # [HID, 512]
        wTk = wTc.reshape(16, 128, 512)
        b_c = bqkv[rows].astype(f32)           # [512]
        b4c = np.ascontiguousarray(b_c.reshape(4, 128).T)  # [128, 4]
        woc = wo[:, 2 * c * HD:(2 * c + 2) * HD]  # [HID, 256]
        woTc = np.ascontiguousarray(woc.T).astype(f32)  # [256, HID]
        woTh = np.ascontiguousarray(
            woTc.reshape(2, 128, HID).transpose(1, 0, 2)).astype(bf16)
        im = {
            "hT": hTh, "b4": b4c, "woT": woTh,
            "cs2": cs2, "ss2": ss2,
            "qw": qw, "qws": qws, "kw": kw, "kws": kws,
            "ones": ones_np, "onesb": onesb_np, "rswap": rt,
            "tmask": tmask,
        }
        for oc in range(4):
            im[f"wT{oc}"] = np.ascontiguousarray(
                wTk[:, :, oc * 128:(oc + 1) * 128].transpose(1, 0, 2)).astype(
                    bf16)
        in_maps.append(im)
    return in_maps


_NC_CACHE = {}


def kernel(hidden_states, cos, sin, k_cache, v_cache, mask,
           wqkv, bqkv, wo, bo, q_norm_w, k_norm_w, kv_write_indices,
           trace=False):
    hidden_states = np.asarray(hidden_states, dtype=np.float32)
    in_maps = _prep_inputs(
        np.asarray(hidden_states), np.asarray(cos), np.asarray(sin),
        np.asarray(wqkv), np.asarray(bqkv), np.asarray(wo),
        np.asarray(q_norm_w), np.asarray(k_norm_w))

    if "nc" not in _NC_CACHE:
        _NC_CACHE["nc"] = build_nc()
    nc = _NC_CACHE["nc"]

    res = run_bass_kernel_spmd(nc, in_maps, core_ids=list(range(N_CORES)),
                               trace=trace)
    out = np.zeros((S, HID), np.float32)
    for rmap in res.results:
        out += np.asarray(rmap["outp"], dtype=np.float32)
    out += np.asarray(bo, dtype=np.float32)[None, :]
    if trace:
        kernel.last_results = res
    return out.reshape(1, S, HID)


# revision 37
# speedup vs baseline: 1.0112x; 1.0112x over previous
"""Trainium2 Bass kernel for nn_Attention_3736621547687.

B=1, S=2048, HID=2048, NH=16, NKV=4, HD=128 attention block:
qkv proj -> per-head RMSNorm(q,k) -> RoPE -> causal GQA attention -> o proj.

Sharding: tensor-parallel over heads across 8 cores. Core c owns q heads
{2c, 2c+1} and kv head c//2 (replicated across the pair of cores sharing it).
Each core computes a partial o-projection output; the host sums the 8
partials (Megatron-style row-parallel reduce) and adds the output bias.

Device-side layout trick: everything is computed in "transposed" orientation
(feature dim on partitions, sequence on the free dim) so that no on-chip
transposes of activations are needed:
  - host supplies hidden^T, wqkv_c^T, wo_c^T, cos/sin tiled to [128, S],
    all pre-converted to bf16 so HBM traffic is halved
  - qkv proj emits q^T/k^T/v^T directly
  - scores are computed as scores^T [keys, queries]; softmax denominators are
    partition-dim sums obtained with an all-ones [128,128] matmul that also
    replicates the result across partitions (giving the broadcast for free)
  - exp() is fused with the 1/sqrt(qpa) scale on the scalar engine; causal
    masking = zeroing exp values above the diagonal with affine_select
    (identical to the reference's additive -1e9 mask in fp32)
  - softmax max-subtraction is skipped: scores are ~N(0,1) after RMSNorm so
    exp() cannot overflow; mathematically identical to the reference.
All matmuls run in bf16 (full PE rate); PSUM accumulation is fp32.
A burst of identity warm-up matmuls at kernel start flips the PE HAM clock
gate to 2.4 GHz while the first weight/activation DMAs are still in flight.
"""

import numpy as np
from contextlib import ExitStack

import concourse.bass as bass
import concourse.bacc as bacc
import concourse.mybir as mybir
import concourse.tile as tile
from concourse.masks import make_identity
from concourse.bass_utils import run_bass_kernel_spmd

S = 2048
HID = 2048
NH = 16
NKV = 4
HD = 128
G = NH // NKV
SCALE = float(128.0 ** -0.5)  # query_pre_attn_scalar = 128
EPS = 1e-6

FP32 = mybir.dt.float32
F32R = mybir.dt.float32r
BF16 = mybir.dt.bfloat16
MULT = mybir.AluOpType.mult
AF = mybir.ActivationFunctionType

N_CORES = 8
SC_ATT = 512    # moving-dim chunk for every stage


def _patch_act_tables():
    """Force Ln and Exp onto the single combined activation-table set so the
    scalar engine never reloads tables when rms-norm and softmax interleave.
    Set ids must keep their positions, so competing sets are emptied rather
    than removed."""
    import concourse.hw_specs as hw_specs
    import concourse.bacc as bacc_mod
    orig = hw_specs.get_activation_tables

    def patched(module_arch):
        t = orig(module_arch)
        for name in ("exp_and_others", "natural_log", "exp_and_friends"):
            if name in t and "natural_log_exp_and_others" in t:
                t[name] = set()
        return t

    bacc_mod.get_activation_tables = patched


def build_nc():
    _patch_act_tables()
    nc = bacc.Bacc()

    hT = nc.dram_tensor("hT", [128, 16, S], BF16, kind="ExternalInput")
    wts_d = [nc.dram_tensor(f"wT{i}", [128, 16, HD], BF16, kind="ExternalInput")
             for i in range(4)]
    b4 = nc.dram_tensor("b4", [128, 4], FP32, kind="ExternalInput")
    woT = nc.dram_tensor("woT", [128, 2, HID], BF16, kind="ExternalInput")
    cs2 = nc.dram_tensor("cs2", [128, S], BF16, kind="ExternalInput")
    ss2 = nc.dram_tensor("ss2", [128, S], BF16, kind="ExternalInput")
    qw = nc.dram_tensor("qw", [128, 1], FP32, kind="ExternalInput")
    qws = nc.dram_tensor("qws", [128, 1], FP32, kind="ExternalInput")
    kw = nc.dram_tensor("kw", [128, 1], FP32, kind="ExternalInput")
    kws = nc.dram_tensor("kws", [128, 1], FP32, kind="ExternalInput")
    onesd = nc.dram_tensor("ones", [128, 128], F32R, kind="ExternalInput")
    onesbd = nc.dram_tensor("onesb", [128, 128], BF16, kind="ExternalInput")
    rswapd = nc.dram_tensor("rswap", [128, 128], BF16, kind="ExternalInput")
    tmaskd = nc.dram_tensor("tmask", [128, 2 * SC_ATT], BF16,
                            kind="ExternalInput")
    outp = nc.dram_tensor("outp", [S, HID], BF16, kind="ExternalOutput")

    with ExitStack() as ctx:
        tc = ctx.enter_context(tile.TileContext(nc))

        const = ctx.enter_context(tc.tile_pool(name="const", bufs=1))
        hpool = ctx.enter_context(tc.tile_pool(name="hpool", bufs=2))
        rawp = ctx.enter_context(tc.tile_pool(name="rawp", bufs=1))
        atp = ctx.enter_context(tc.tile_pool(name="atp", bufs=1))
        wpool = ctx.enter_context(tc.tile_pool(name="wpool", bufs=3))
        vpool = ctx.enter_context(tc.tile_pool(name="vpool", bufs=1))
        ppool = ctx.enter_context(tc.tile_pool(name="ppool", bufs=2))
        dpool = ctx.enter_context(tc.tile_pool(name="dpool", bufs=2))
        opool = ctx.enter_context(tc.tile_pool(name="opool", bufs=2))

        psA = ctx.enter_context(tc.tile_pool(name="psA", bufs=1, space="PSUM"))
        psB = ctx.enter_context(tc.tile_pool(name="psB", bufs=2, space="PSUM"))
        psC = ctx.enter_context(tc.tile_pool(name="psC", bufs=3, space="PSUM"))
        psO = ctx.enter_context(tc.tile_pool(name="psO", bufs=2, space="PSUM"))

        # ---- identity first: it feeds the PE warm-up matmuls ----------
        ident = const.tile([128, 128], BF16)
        make_identity(nc, ident)

        # ---- weight loads on the sync HWDGE queue (k-head slice first) -
        b4s = const.tile([128, 4], FP32)
        nc.sync.dma_start(out=b4s, in_=b4[:, :])
        wts = [None] * 4
        for oc in (2, 0, 1, 3):
            wt = const.tile([128, 16, HD], BF16, name=f"wts{oc}",
                            tag=f"wts{oc}")
            # split halves so the first proj group can start on kt 0-7
            # while the rest of the weights stream in
            nc.sync.dma_start(out=wt[:, 0:8, :], in_=wts_d[oc][:, 0:8, :])
            nc.sync.dma_start(out=wt[:, 8:16, :], in_=wts_d[oc][:, 8:16, :])
            wts[oc] = wt

        # ---- small constants on the SWDGE queue ------------------------
        ones128 = const.tile([128, 128], F32R)
        nc.gpsimd.dma_start(out=ones128, in_=onesd[:, :])
        onesb = const.tile([128, 128], BF16)
        nc.gpsimd.dma_start(out=onesb, in_=onesbd[:, :])
        Rm = const.tile([128, 128], BF16)
        nc.gpsimd.dma_start(out=Rm, in_=rswapd[:, :])
        tmask = const.tile([128, 2 * SC_ATT], BF16)
        nc.gpsimd.dma_start(out=tmask, in_=tmaskd[:, :])
        qw_ = const.tile([128, 1], FP32)
        nc.gpsimd.dma_start(out=qw_, in_=qw[:, :])
        qws_ = const.tile([128, 1], FP32)
        nc.gpsimd.dma_start(out=qws_, in_=qws[:, :])
        kw_ = const.tile([128, 1], FP32)
        nc.gpsimd.dma_start(out=kw_, in_=kw[:, :])
        kws_ = const.tile([128, 1], FP32)
        nc.gpsimd.dma_start(out=kws_, in_=kws[:, :])
        epsc = const.tile([128, 1], FP32)
        nc.vector.memset(epsc, EPS)

        # ---- PE warm-up: ~4us of dummy matmuls so the HAM clock gate is
        # at 2.4 GHz by the time the first weight DMA lands ---------------
        warm = psC.tile([128, SC_ATT], FP32, tag="score")
        for _ in range(40):
            nc.tensor.matmul(warm[:, 0:128], lhsT=ident, rhs=ident,
                             start=True, stop=True)

        rawq = [rawp.tile([128, S], BF16, tag=f"raw{i}", name=f"raw{i}")
                for i in range(3)]
        qhat = [atp.tile([128, S], BF16, tag=f"qh{i}", name=f"qh{i}")
                for i in range(2)]
        khat = atp.tile([128, S], BF16, tag="kh")
        attnT = [atp.tile([128, S], BF16, tag=f"attnT{h}", name=f"attnT{h}")
                 for h in range(2)]
        vsb = vpool.tile([128, 16, HD], BF16, tag="vsb")
        heads = [
            (rawq[2], khat, kw_, kws_),
            (rawq[0], qhat[0], qw_, qws_),
            (rawq[1], qhat[1], qw_, qws_),
        ]
        raw3s = {}
        pending_norm = []

        def emit_norm(item):
            outps_, dacc_, h_, sl_ = item
            drep = psC.tile([128, SC_ATT], FP32, tag="score")
            nc.tensor.matmul(drep, lhsT=ones128, rhs=dacc_,
                             start=True, stop=True)
            drec = wpool.tile([128, SC_ATT], FP32, tag="tt", bufs=3)
            nc.vector.reciprocal_approx_fast(drec, drep)
            nc.vector.tensor_mul(attnT[h_][:, sl_], outps_, drec)

        # ================================================================
        # Software-pipelined emission: stage lag guarantees every
        # instruction's inputs are a full pipeline iteration old, so no
        # engine stream ever blocks at a phase boundary.
        #   iter sc: proj(sc) | rope+V(sc-1) | attention(sc-2) | oproj(sc-3)
        # ================================================================
        NCH = S // SC_ATT
        for it in range(NCH + 3):
            # ---- stage 1: qkv projection ------------------------------
            if it < NCH:
                sc = it
                sl = bass.ts(sc, SC_ATT)
                htsA = hpool.tile([128, 8, SC_ATT], BF16, tag="htsA")
                htsB = hpool.tile([128, 8, SC_ATT], BF16, tag="htsB")
                if sc == 0:
                    # fine-grained first load: earliest possible proj start
                    for q in range(4):
                        dst = htsA if q < 2 else htsB
                        nc.scalar.dma_start(
                            out=dst[:, 4 * (q % 2):4 * (q % 2) + 4, :],
                            in_=hT[:, 4 * q:4 * q + 4, sl])
                else:
                    nc.scalar.dma_start(out=htsA, in_=hT[:, 0:8, sl])
                    nc.scalar.dma_start(out=htsB, in_=hT[:, 8:16, sl])
                if sc == 1:
                    # deferred constants ride the scalar queue behind the
                    # chunk-1 activations: cos/sin first needed by rope(0)
                    # at ~25us, o-proj weights at iteration 3
                    cs2s = const.tile([128, S], BF16)
                    nc.scalar.dma_start(out=cs2s, in_=cs2[:, :])
                    ss2s = const.tile([128, S], BF16)
                    nc.scalar.dma_start(out=ss2s, in_=ss2[:, :])
                    woTs = const.tile([128, 2, HID], BF16)
                    nc.scalar.dma_start(out=woTs, in_=woT[:, :, :])
                for oc in (2, 0, 1, 3):
                    ps = psA.tile([128, SC_ATT], FP32, tag="mm")
                    for kt in range(16):
                        src_h = htsA if kt < 8 else htsB
                        nc.tensor.matmul(
                            ps, lhsT=wts[oc][:, kt, :],
                            rhs=src_h[:, kt % 8, :],
                            start=(kt == 0), stop=(kt == 15))
                    if oc == 3:
                        raw3 = wpool.tile([128, SC_ATT], BF16, tag="raw3",
                                          bufs=2)
                        nc.vector.tensor_scalar_add(raw3, ps,
                                                    b4s[:, oc:oc + 1])
                        raw3s[sc] = raw3
                    else:
                        nc.vector.tensor_scalar_add(
                            rawq[oc][:, sl], ps, b4s[:, oc:oc + 1])

            # ---- stage 2: rmsnorm + rope + V tiles --------------------
            if 1 <= it <= NCH:
                sc = it - 1
                sl = bass.ts(sc, SC_ATT)
                for raw, dst, w_, wsw_ in heads:
                    sq = wpool.tile([128, SC_ATT], BF16, tag="sq", bufs=2)
                    nc.vector.tensor_mul(sq, raw[:, sl], raw[:, sl])
                    ssum = psC.tile([128, SC_ATT], FP32, tag="score")
                    nc.tensor.matmul(ssum, lhsT=onesb, rhs=sq,
                                     start=True, stop=True)
                    lnb = wpool.tile([128, SC_ATT], FP32, tag="lnb", bufs=2)
                    nc.scalar.activation(lnb, ssum, AF.Ln,
                                         scale=1.0 / HD, bias=epsc)
                    nc.scalar.activation(lnb, lnb, AF.Exp, scale=-0.5)
                    rtp = psC.tile([128, SC_ATT], FP32, tag="score")
                    nc.tensor.matmul(rtp, lhsT=Rm, rhs=raw[:, sl],
                                     start=True, stop=True)
                    t1 = wpool.tile([128, SC_ATT], BF16, tag="tt", bufs=3)
                    nc.vector.scalar_tensor_tensor(
                        t1, in0=raw[:, sl], scalar=w_, in1=cs2s[:, sl],
                        op0=MULT, op1=MULT)
                    t2 = wpool.tile([128, SC_ATT], BF16, tag="tt", bufs=3)
                    nc.vector.scalar_tensor_tensor(
                        t2, in0=rtp, scalar=wsw_, in1=ss2s[:, sl],
                        op0=MULT, op1=MULT)
                    t3 = wpool.tile([128, SC_ATT], BF16, tag="tt", bufs=3)
                    nc.vector.tensor_add(t3, t1, t2)
                    nc.vector.tensor_mul(dst[:, sl], t3, lnb)
                raw3 = raw3s.pop(sc)
                for j in range(4):
                    tt = 4 * sc + j
                    vps = psC.tile([128, SC_ATT], BF16, tag="score")
                    nc.tensor.transpose(vps[:, 0:128],
                                        raw3[:, bass.ts(j, 128)], ident)
                    nc.vector.tensor_copy(vsb[:, tt, :], vps[:, 0:128])

            # ---- stage 3: attention, both heads -----------------------
            if 2 <= it <= NCH + 1:
                sc = it - 2
                sl = bass.ts(sc, SC_ATT)
                s0 = sc * SC_ATT
                ntt = sc * 4 + 4
                # previous chunk's softmax normalizations are emitted first:
                # their ones-matmuls (which wait on the vector dacc chain)
                # sit behind this chunk's score matmuls in the tensor FIFO
                # instead of stalling it, and they release the psO slots
                # this chunk's PV accumulations will take over
                while pending_norm:
                    emit_norm(pending_norm.pop(0))
                outps_ = [psO.tile([128, SC_ATT], FP32, tag="attnout",
                                   name=f"outps{hh}") for hh in range(2)]
                dacc_ = [dpool.tile([128, SC_ATT], F32R, tag="dacc",
                                    bufs=4, name=f"dacc{hh}")
                         for hh in range(2)]
                for g in range(ntt // 4):
                    for h in range(2):
                        outps = outps_[h]
                        dacc = dacc_[h]
                        pg = ppool.tile([128, 4, SC_ATT], BF16, tag="pt",
                                        bufs=4)
                        for j in range(4):
                            tt = g * 4 + j
                            band = tt >= sc * 4  # diagonal band: mask t > s
                            scp = psC.tile([128, SC_ATT], FP32, tag="score")
                            nc.tensor.matmul(
                                scp, lhsT=khat[:, bass.ts(tt, 128)],
                                rhs=qhat[h][:, sl],
                                start=True, stop=not band)
                            if band:
                                # additive causal mask: -1e9 where q < k,
                                # identical to the reference's mask add.
                                # tmask[p, SC+c] = -1e9*[c < p]; column c
                                # of this tile is global query s0+c vs key
                                # 128*tt+p  ->  c_local = c - 128*(tt-4*sc)
                                off = SC_ATT - 128 * (tt - 4 * sc)
                                nc.tensor.matmul(
                                    scp, lhsT=ident,
                                    rhs=tmask[:, bass.ds(off, SC_ATT)],
                                    start=False, stop=True)
                            nc.scalar.activation(pg[:, j, :], scp, AF.Exp,
                                                 scale=SCALE)
                            nc.tensor.matmul(outps, lhsT=vsb[:, tt, :],
                                             rhs=pg[:, j, :],
                                             start=(tt == 0),
                                             stop=(tt == ntt - 1))
                        # group reduction split across vector + gpsimd
                        ga = ppool.tile([128, SC_ATT], BF16, tag="ga",
                                        bufs=3)
                        nc.vector.tensor_add(ga, pg[:, 0, :], pg[:, 1, :])
                        gb = ppool.tile([128, SC_ATT], BF16, tag="gb",
                                        bufs=3)
                        nc.gpsimd.tensor_add(gb, pg[:, 2, :], pg[:, 3, :])
                        if g == 0:
                            nc.vector.tensor_add(dacc, ga, gb)
                        else:
                            gc = ppool.tile([128, SC_ATT], BF16, tag="ga",
                                            bufs=3)
                            nc.vector.tensor_add(gc, ga, gb)
                            nc.vector.tensor_add(dacc, dacc, gc)
                for h in range(2):
                    pending_norm.append((outps_[h], dacc_[h], h, sl))

            # ---- stage 4: o projection --------------------------------
            if it >= 3:
                sc = it - 3
                if sc == NCH - 1:
                    while pending_norm:
                        emit_norm(pending_norm.pop(0))
                ti = 0
                for st in range(4 * sc, 4 * sc + 4):
                    for jc in range(HID // SC_ATT):
                        if sc == NCH - 1:
                            # drain: proj + score PSUM banks are free now;
                            # rotate through them for 5-deep buffering
                            pool, tg = [(psB, "omm"), (psA, "mm"),
                                        (psC, "score")][ti % 3]
                            ops = pool.tile([128, SC_ATT], FP32, tag=tg)
                        elif sc >= 1:
                            # qkv proj is done by iteration 4: its PSUM
                            # bank joins the o-proj rotation (3 slots)
                            pool, tg = [(psB, "omm"), (psB, "omm"),
                                        (psA, "mm")][ti % 3]
                            ops = pool.tile([128, SC_ATT], FP32, tag=tg)
                        else:
                            ops = psB.tile([128, SC_ATT], FP32, tag="omm")
                        ti += 1
                        for h in range(2):
                            nc.tensor.matmul(
                                ops, lhsT=attnT[h][:, bass.ts(st, 128)],
                                rhs=woTs[:, h, bass.ts(jc, SC_ATT)],
                                start=(h == 0), stop=(h == 1))
                        osb = opool.tile([128, SC_ATT], BF16, tag="osb",
                                         bufs=4)
                        # halve the PSUM-bank hold time: both engines copy
                        # one half of the tile in parallel
                        nc.vector.tensor_copy(osb[:, 0:256], ops[:, 0:256])
                        nc.scalar.copy(osb[:, 256:512], ops[:, 256:512])
                        nc.sync.dma_start(
                            out=outp[bass.ts(st, 128), bass.ts(jc, SC_ATT)],
                            in_=osb)

    nc.compile()
    return nc


def _prep_inputs(hidden_states, cos, sin, wqkv, bqkv, wo, q_norm_w, k_norm_w):
    """Host-side layout prep + per-core sharding. All device tensors are
    pre-swizzled (and converted to bf16) so every DMA has long contiguous
    per-partition runs at half the fp32 byte count."""
    import ml_dtypes
    f32 = np.float32
    bf16 = ml_dtypes.bfloat16
    hTn = np.ascontiguousarray(hidden_states.reshape(S, HID).T).astype(f32)
    hTh = np.ascontiguousarray(
        hTn.reshape(16, 128, S).transpose(1, 0, 2)).astype(bf16)  # [p, kt, s]
    cosT = cos.T.astype(f32)  # [64, S]
    sinT = sin.T.astype(f32)
    cs2 = np.ascontiguousarray(
        np.concatenate([cosT, cosT], axis=0)).astype(bf16)
    ss2 = np.ascontiguousarray(
        np.concatenate([sinT, sinT], axis=0)).astype(bf16)
    qw = np.ascontiguousarray(q_norm_w.reshape(128, 1)).astype(f32)
    qws = np.ascontiguousarray(
        np.concatenate([q_norm_w[64:], q_norm_w[:64]]).reshape(128, 1)).astype(f32)
    kw = np.ascontiguousarray(k_norm_w.reshape(128, 1)).astype(f32)
    kws = np.ascontiguousarray(
        np.concatenate([k_norm_w[64:], k_norm_w[:64]]).reshape(128, 1)).astype(f32)
    ones_np = np.ones((128, 128), dtype=f32)
    onesb_np = np.ones((128, 128), dtype=bf16)
    rt = np.zeros((128, 128), dtype=f32)
    rt[np.arange(64) + 64, np.arange(64)] = -1.0   # R^T[d+64, d] = -1
    rt[np.arange(64), np.arange(64) + 64] = 1.0    # R^T[d-64, d] = +1
    rt = rt.astype(bf16)
    # causal mask template: tmask[p, 512+d] = -1e9 where d < p
    dcol = np.arange(2 * 512)[None, :] - 512
    prow = np.arange(128)[:, None]
    tmask = np.where(dcol < prow, np.float32(-1e9), np.float32(0.0)).astype(bf16)

    in_maps = []
    for c in range(N_CORES):
        kvh = c // 2
        rows = list(range(2 * c * HD, (2 * c + 2) * HD))          # q0, q1
        rows += list(range(NH * HD + kvh * HD, NH * HD + (kvh + 1) * HD))  # k
        rows += list(range((NH + NKV) * HD + kvh * HD,
                           (NH + NKV) * HD + (kvh + 1) * HD))      # v
        w_c = wqkv[rows]                       # [512, HID]
        wTc = np.ascontiguousarray(w_c.T).astype(f32)   # [HID, 512]
        wTk = wTc.reshape(16, 128, 512)
        b_c = bqkv[rows].astype(f32)           # [512]
        b4c = np.ascontiguousarray(b_c.reshape(4, 128).T)  # [128, 4]
        woc = wo[:, 2 * c * HD:(2 * c + 2) * HD]  # [HID, 256]
        woTc = np.ascontiguousarray(woc.T).astype(f32)  # [256, HID]
        woTh = np.ascontiguousarray(
            woTc.reshape(2, 128, HID).transpose(1, 0, 2)).astype(bf16)
        im = {
            "hT": hTh, "b4": b4c, "woT": woTh,
            "cs2": cs2, "ss2": ss2,
            "qw": qw, "qws": qws, "kw": kw, "kws": kws,
            "ones": ones_np, "onesb": onesb_np, "rswap": rt,
            "tmask": tmask,
        }
        for oc in range(4):
            im[f"wT{oc}"] = np.ascontiguousarray(
                wTk[:, :, oc * 128:(oc + 1) * 128].transpose(1, 0, 2)).astype(
                    bf16)
        in_maps.append(im)
    return in_maps


_NC_CACHE = {}


def kernel(hidden_states, cos, sin, k_cache, v_cache, mask,
           wqkv, bqkv, wo, bo, q_norm_w, k_norm_w, kv_write_indices,
           trace=False):
    hidden_states = np.asarray(hidden_states, dtype=np.float32)
    in_maps = _prep_inputs(
        np.asarray(hidden_states), np.asarray(cos), np.asarray(sin),
        np.asarray(wqkv), np.asarray(bqkv), np.asarray(wo),
        np.asarray(q_norm_w), np.asarray(k_norm_w))

    if "nc" not in _NC_CACHE:
        _NC_CACHE["nc"] = build_nc()
    nc = _NC_CACHE["nc"]

    res = run_bass_kernel_spmd(nc, in_maps, core_ids=list(range(N_CORES)),
                               trace=trace)
    out = np.zeros((S, HID), np.float32)
    for rmap in res.results:
        out += np.asarray(rmap["outp"], dtype=np.float32)
    out += np.asarray(bo, dtype=np.float32)[None, :]
    if trace:
        kernel.last_results = res
    return out.reshape(1, S, HID)


# revision 39
# speedup vs baseline: 1.0328x; 1.0214x over previous
"""Trainium2 Bass kernel for nn_Attention_3736621547687.

B=1, S=2048, HID=2048, NH=16, NKV=4, HD=128 attention block:
qkv proj -> per-head RMSNorm(q,k) -> RoPE -> causal GQA attention -> o proj.

Sharding: tensor-parallel over heads across 8 cores. Core c owns q heads
{2c, 2c+1} and kv head c//2 (replicated across the pair of cores sharing it).
Each core computes a partial o-projection output; the host sums the 8
partials (Megatron-style row-parallel reduce) and adds the output bias.

Device-side layout trick: everything is computed in "transposed" orientation
(feature dim on partitions, sequence on the free dim) so that no on-chip
transposes of activations are needed:
  - host supplies hidden^T, wqkv_c^T, wo_c^T, cos/sin tiled to [128, S],
    all pre-converted to bf16 so HBM traffic is halved
  - qkv proj emits q^T/k^T/v^T directly
  - scores are computed as scores^T [keys, queries]; softmax denominators are
    partition-dim sums obtained with an all-ones [128,128] matmul that also
    replicates the result across partitions (giving the broadcast for free)
  - exp() is fused with the 1/sqrt(qpa) scale on the scalar engine; causal
    masking = zeroing exp values above the diagonal with affine_select
    (identical to the reference's additive -1e9 mask in fp32)
  - softmax max-subtraction is skipped: scores are ~N(0,1) after RMSNorm so
    exp() cannot overflow; mathematically identical to the reference.
All matmuls run in bf16 (full PE rate); PSUM accumulation is fp32.
A burst of identity warm-up matmuls at kernel start flips the PE HAM clock
gate to 2.4 GHz while the first weight/activation DMAs are still in flight.
"""

import numpy as np
from contextlib import ExitStack

import concourse.bass as bass
import concourse.bacc as bacc
import concourse.mybir as mybir
import concourse.tile as tile
from concourse.masks import make_identity
from concourse.bass_utils import run_bass_kernel_spmd

S = 2048
HID = 2048
NH = 16
NKV = 4
HD = 128
G = NH // NKV
SCALE = float(128.0 ** -0.5)  # query_pre_attn_scalar = 128
EPS = 1e-6

FP32 = mybir.dt.float32
F32R = mybir.dt.float32r
BF16 = mybir.dt.bfloat16
MULT = mybir.AluOpType.mult
AF = mybir.ActivationFunctionType

N_CORES = 8
SC_ATT = 512    # moving-dim chunk for every stage


def _patch_act_tables():
    """Force Ln and Exp onto the single combined activation-table set so the
    scalar engine never reloads tables when rms-norm and softmax interleave.
    Set ids must keep their positions, so competing sets are emptied rather
    than removed."""
    import concourse.hw_specs as hw_specs
    import concourse.bacc as bacc_mod
    orig = hw_specs.get_activation_tables

    def patched(module_arch):
        t = orig(module_arch)
        for name in ("exp_and_others", "natural_log", "exp_and_friends"):
            if name in t and "natural_log_exp_and_others" in t:
                t[name] = set()
        return t

    bacc_mod.get_activation_tables = patched


def build_nc():
    _patch_act_tables()
    nc = bacc.Bacc()

    hT = nc.dram_tensor("hT", [128, 16, S], BF16, kind="ExternalInput")
    wts_d = [nc.dram_tensor(f"wT{i}", [128, 16, HD], BF16, kind="ExternalInput")
             for i in range(4)]
    b4 = nc.dram_tensor("b4", [128, 4], FP32, kind="ExternalInput")
    woT = nc.dram_tensor("woT", [128, 2, HID], BF16, kind="ExternalInput")
    cs2 = nc.dram_tensor("cs2", [128, S], BF16, kind="ExternalInput")
    ss2 = nc.dram_tensor("ss2", [128, S], BF16, kind="ExternalInput")
    qw = nc.dram_tensor("qw", [128, 1], FP32, kind="ExternalInput")
    qws = nc.dram_tensor("qws", [128, 1], FP32, kind="ExternalInput")
    kw = nc.dram_tensor("kw", [128, 1], FP32, kind="ExternalInput")
    kws = nc.dram_tensor("kws", [128, 1], FP32, kind="ExternalInput")
    onesd = nc.dram_tensor("ones", [128, 128], F32R, kind="ExternalInput")
    onesbd = nc.dram_tensor("onesb", [128, 128], BF16, kind="ExternalInput")
    rswapd = nc.dram_tensor("rswap", [128, 128], BF16, kind="ExternalInput")
    tmaskd = nc.dram_tensor("tmask", [128, 2 * SC_ATT], BF16,
                            kind="ExternalInput")
    outp = nc.dram_tensor("outp", [S, HID], BF16, kind="ExternalOutput")

    with ExitStack() as ctx:
        tc = ctx.enter_context(tile.TileContext(nc))

        const = ctx.enter_context(tc.tile_pool(name="const", bufs=1))
        hpool = ctx.enter_context(tc.tile_pool(name="hpool", bufs=2))
        rawp = ctx.enter_context(tc.tile_pool(name="rawp", bufs=1))
        atp = ctx.enter_context(tc.tile_pool(name="atp", bufs=1))
        wpool = ctx.enter_context(tc.tile_pool(name="wpool", bufs=3))
        vpool = ctx.enter_context(tc.tile_pool(name="vpool", bufs=1))
        ppool = ctx.enter_context(tc.tile_pool(name="ppool", bufs=2))
        dpool = ctx.enter_context(tc.tile_pool(name="dpool", bufs=2))
        opool = ctx.enter_context(tc.tile_pool(name="opool", bufs=2))

        psA = ctx.enter_context(tc.tile_pool(name="psA", bufs=1, space="PSUM"))
        psB = ctx.enter_context(tc.tile_pool(name="psB", bufs=2, space="PSUM"))
        psC = ctx.enter_context(tc.tile_pool(name="psC", bufs=3, space="PSUM"))
        psO = ctx.enter_context(tc.tile_pool(name="psO", bufs=2, space="PSUM"))

        # ---- identity first: it feeds the PE warm-up matmuls ----------
        ident = const.tile([128, 128], BF16)
        make_identity(nc, ident)

        # ---- startup loads split across BOTH HWDGE queues so the first
        # proj group's inputs (w2 + hts0) land as early as possible; the
        # remaining weights stream behind hts0B on the scalar queue -------
        b4s = const.tile([128, 4], FP32)
        nc.sync.dma_start(out=b4s, in_=b4[:, :])
        wts = [None] * 4
        for oc in range(4):
            wts[oc] = const.tile([128, 16, HD], BF16, name=f"wts{oc}",
                                 tag=f"wts{oc}")
        nc.scalar.dma_start(out=wts[2][:, 0:8, :], in_=wts_d[2][:, 0:8, :])
        nc.scalar.dma_start(out=wts[2][:, 8:16, :], in_=wts_d[2][:, 8:16, :])

        # ---- small constants on the SWDGE queue ------------------------
        ones128 = const.tile([128, 128], F32R)
        nc.gpsimd.dma_start(out=ones128, in_=onesd[:, :])
        onesb = const.tile([128, 128], BF16)
        nc.gpsimd.dma_start(out=onesb, in_=onesbd[:, :])
        Rm = const.tile([128, 128], BF16)
        nc.gpsimd.dma_start(out=Rm, in_=rswapd[:, :])
        tmask = const.tile([128, 2 * SC_ATT], BF16)
        nc.gpsimd.dma_start(out=tmask, in_=tmaskd[:, :])
        qw_ = const.tile([128, 1], FP32)
        nc.gpsimd.dma_start(out=qw_, in_=qw[:, :])
        qws_ = const.tile([128, 1], FP32)
        nc.gpsimd.dma_start(out=qws_, in_=qws[:, :])
        kw_ = const.tile([128, 1], FP32)
        nc.gpsimd.dma_start(out=kw_, in_=kw[:, :])
        kws_ = const.tile([128, 1], FP32)
        nc.gpsimd.dma_start(out=kws_, in_=kws[:, :])
        epsc = const.tile([128, 1], FP32)
        nc.vector.memset(epsc, EPS)

        # ---- PE warm-up: ~4us of dummy matmuls so the HAM clock gate is
        # at 2.4 GHz by the time the first weight DMA lands ---------------
        warm = psC.tile([128, SC_ATT], FP32, tag="score")
        for _ in range(40):
            nc.tensor.matmul(warm[:, 0:128], lhsT=ident, rhs=ident,
                             start=True, stop=True)

        rawq = [rawp.tile([128, S], BF16, tag=f"raw{i}", name=f"raw{i}")
                for i in range(3)]
        qhat = [atp.tile([128, S], BF16, tag=f"qh{i}", name=f"qh{i}")
                for i in range(2)]
        khat = atp.tile([128, S], BF16, tag="kh")
        attnT = [atp.tile([128, S], BF16, tag=f"attnT{h}", name=f"attnT{h}")
                 for h in range(2)]
        vsb = vpool.tile([128, 16, HD], BF16, tag="vsb")
        heads = [
            (rawq[2], khat, kw_, kws_),
            (rawq[0], qhat[0], qw_, qws_),
            (rawq[1], qhat[1], qw_, qws_),
        ]
        raw3s = {}
        pending_norm = []

        def emit_norm(item):
            outps_, dacc_, h_, sl_ = item
            drep = psC.tile([128, SC_ATT], FP32, tag="score")
            nc.tensor.matmul(drep, lhsT=ones128, rhs=dacc_,
                             start=True, stop=True)
            drec = wpool.tile([128, SC_ATT], FP32, tag="tt", bufs=3)
            nc.vector.reciprocal_approx_fast(drec, drep)
            nc.vector.tensor_mul(attnT[h_][:, sl_], outps_, drec)

        # ================================================================
        # Software-pipelined emission: stage lag guarantees every
        # instruction's inputs are a full pipeline iteration old, so no
        # engine stream ever blocks at a phase boundary.
        #   iter sc: proj(sc) | rope+V(sc-1) | attention(sc-2) | oproj(sc-3)
        # ================================================================
        NCH = S // SC_ATT
        for it in range(NCH + 3):
            # ---- stage 1: qkv projection ------------------------------
            if it < NCH:
                sc = it
                sl = bass.ts(sc, SC_ATT)
                htsA = hpool.tile([128, 8, SC_ATT], BF16, tag="htsA")
                htsB = hpool.tile([128, 8, SC_ATT], BF16, tag="htsB")
                if sc == 0:
                    # chunk-0 activations split across both queues; the
                    # remaining qkv weights follow hts0B on scalar
                    nc.sync.dma_start(out=htsA[:, 0:4, :],
                                      in_=hT[:, 0:4, sl])
                    nc.sync.dma_start(out=htsA[:, 4:8, :],
                                      in_=hT[:, 4:8, sl])
                    nc.scalar.dma_start(out=htsB[:, 0:4, :],
                                        in_=hT[:, 8:12, sl])
                    nc.scalar.dma_start(out=htsB[:, 4:8, :],
                                        in_=hT[:, 12:16, sl])
                    for oc in (0, 1, 3):
                        nc.scalar.dma_start(out=wts[oc][:, 0:8, :],
                                            in_=wts_d[oc][:, 0:8, :])
                        nc.scalar.dma_start(out=wts[oc][:, 8:16, :],
                                            in_=wts_d[oc][:, 8:16, :])
                else:
                    nc.scalar.dma_start(out=htsA, in_=hT[:, 0:8, sl])
                    nc.scalar.dma_start(out=htsB, in_=hT[:, 8:16, sl])
                if sc == 1:
                    # deferred constants ride the scalar queue behind the
                    # chunk-1 activations: cos/sin first needed by rope(0)
                    # at ~25us, o-proj weights at iteration 3
                    cs2s = const.tile([128, S], BF16)
                    nc.scalar.dma_start(out=cs2s, in_=cs2[:, :])
                    ss2s = const.tile([128, S], BF16)
                    nc.scalar.dma_start(out=ss2s, in_=ss2[:, :])
                    woTs = const.tile([128, 2, HID], BF16)
                    nc.scalar.dma_start(out=woTs, in_=woT[:, :, :])
                for oc in (2, 0, 1, 3):
                    ps = psA.tile([128, SC_ATT], FP32, tag="mm")
                    for kt in range(16):
                        src_h = htsA if kt < 8 else htsB
                        nc.tensor.matmul(
                            ps, lhsT=wts[oc][:, kt, :],
                            rhs=src_h[:, kt % 8, :],
                            start=(kt == 0), stop=(kt == 15))
                    if oc == 3:
                        raw3 = wpool.tile([128, SC_ATT], BF16, tag="raw3",
                                          bufs=2)
                        nc.vector.tensor_scalar_add(raw3, ps,
                                                    b4s[:, oc:oc + 1])
                        raw3s[sc] = raw3
                    else:
                        nc.vector.tensor_scalar_add(
                            rawq[oc][:, sl], ps, b4s[:, oc:oc + 1])

            # ---- stage 2: rmsnorm + rope + V tiles --------------------
            if 1 <= it <= NCH:
                sc = it - 1
                sl = bass.ts(sc, SC_ATT)
                for raw, dst, w_, wsw_ in heads:
                    sq = wpool.tile([128, SC_ATT], BF16, tag="sq", bufs=2)
                    nc.vector.tensor_mul(sq, raw[:, sl], raw[:, sl])
                    ssum = psC.tile([128, SC_ATT], FP32, tag="score")
                    nc.tensor.matmul(ssum, lhsT=onesb, rhs=sq,
                                     start=True, stop=True)
                    lnb = wpool.tile([128, SC_ATT], FP32, tag="lnb", bufs=2)
                    nc.scalar.activation(lnb, ssum, AF.Ln,
                                         scale=1.0 / HD, bias=epsc)
                    nc.scalar.activation(lnb, lnb, AF.Exp, scale=-0.5)
                    rtp = psC.tile([128, SC_ATT], FP32, tag="score")
                    nc.tensor.matmul(rtp, lhsT=Rm, rhs=raw[:, sl],
                                     start=True, stop=True)
                    t1 = wpool.tile([128, SC_ATT], BF16, tag="tt", bufs=3)
                    nc.vector.scalar_tensor_tensor(
                        t1, in0=raw[:, sl], scalar=w_, in1=cs2s[:, sl],
                        op0=MULT, op1=MULT)
                    t2 = wpool.tile([128, SC_ATT], BF16, tag="tt", bufs=3)
                    nc.vector.scalar_tensor_tensor(
                        t2, in0=rtp, scalar=wsw_, in1=ss2s[:, sl],
                        op0=MULT, op1=MULT)
                    t3 = wpool.tile([128, SC_ATT], BF16, tag="tt", bufs=3)
                    nc.vector.tensor_add(t3, t1, t2)
                    nc.vector.tensor_mul(dst[:, sl], t3, lnb)
                raw3 = raw3s.pop(sc)
                for j in range(4):
                    tt = 4 * sc + j
                    vps = psC.tile([128, SC_ATT], BF16, tag="score")
                    nc.tensor.transpose(vps[:, 0:128],
                                        raw3[:, bass.ts(j, 128)], ident)
                    nc.vector.tensor_copy(vsb[:, tt, :], vps[:, 0:128])

            # ---- stage 3: attention, both heads -----------------------
            if 2 <= it <= NCH + 1:
                sc = it - 2
                sl = bass.ts(sc, SC_ATT)
                s0 = sc * SC_ATT
                ntt = sc * 4 + 4
                # previous chunk's softmax normalizations are emitted first:
                # their ones-matmuls (which wait on the vector dacc chain)
                # sit behind this chunk's score matmuls in the tensor FIFO
                # instead of stalling it, and they release the psO slots
                # this chunk's PV accumulations will take over
                while pending_norm:
                    emit_norm(pending_norm.pop(0))
                outps_ = [psO.tile([128, SC_ATT], FP32, tag="attnout",
                                   name=f"outps{hh}") for hh in range(2)]
                dacc_ = [dpool.tile([128, SC_ATT], F32R, tag="dacc",
                                    bufs=4, name=f"dacc{hh}")
                         for hh in range(2)]
                for g in range(ntt // 4):
                    for h in range(2):
                        outps = outps_[h]
                        dacc = dacc_[h]
                        pg = ppool.tile([128, 4, SC_ATT], BF16, tag="pt",
                                        bufs=4)
                        for j in range(4):
                            tt = g * 4 + j
                            band = tt >= sc * 4  # diagonal band: mask t > s
                            scp = psC.tile([128, SC_ATT], FP32, tag="score")
                            nc.tensor.matmul(
                                scp, lhsT=khat[:, bass.ts(tt, 128)],
                                rhs=qhat[h][:, sl],
                                start=True, stop=not band)
                            if band:
                                # additive causal mask: -1e9 where q < k,
                                # identical to the reference's mask add.
                                # tmask[p, SC+c] = -1e9*[c < p]; column c
                                # of this tile is global query s0+c vs key
                                # 128*tt+p  ->  c_local = c - 128*(tt-4*sc)
                                off = SC_ATT - 128 * (tt - 4 * sc)
                                nc.tensor.matmul(
                                    scp, lhsT=ident,
                                    rhs=tmask[:, bass.ds(off, SC_ATT)],
                                    start=False, stop=True)
                            nc.scalar.activation(pg[:, j, :], scp, AF.Exp,
                                                 scale=SCALE)
                            nc.tensor.matmul(outps, lhsT=vsb[:, tt, :],
                                             rhs=pg[:, j, :],
                                             start=(tt == 0),
                                             stop=(tt == ntt - 1))
                        # group reduction split across vector + gpsimd
                        ga = ppool.tile([128, SC_ATT], BF16, tag="ga",
                                        bufs=3)
                        nc.vector.tensor_add(ga, pg[:, 0, :], pg[:, 1, :])
                        gb = ppool.tile([128, SC_ATT], BF16, tag="gb",
                                        bufs=3)
                        nc.gpsimd.tensor_add(gb, pg[:, 2, :], pg[:, 3, :])
                        if g == 0:
                            nc.vector.tensor_add(dacc, ga, gb)
                        else:
                            gc = ppool.tile([128, SC_ATT], BF16, tag="ga",
                                            bufs=3)
                            nc.vector.tensor_add(gc, ga, gb)
                            nc.vector.tensor_add(dacc, dacc, gc)
                for h in range(2):
                    pending_norm.append((outps_[h], dacc_[h], h, sl))

            # ---- stage 4: o projection --------------------------------
            if it >= 3:
                sc = it - 3
                if sc == NCH - 1:
                    while pending_norm:
                        emit_norm(pending_norm.pop(0))
                ti = 0
                for st in range(4 * sc, 4 * sc + 4):
                    for jc in range(HID // SC_ATT):
                        if sc == NCH - 1:
                            # drain: proj + score PSUM banks are free now;
                            # rotate through them for 5-deep buffering
                            pool, tg = [(psB, "omm"), (psA, "mm"),
                                        (psC, "score")][ti % 3]
                            ops = pool.tile([128, SC_ATT], FP32, tag=tg)
                        elif sc >= 1:
                            # qkv proj is done by iteration 4: its PSUM
                            # bank joins the o-proj rotation (3 slots)
                            pool, tg = [(psB, "omm"), (psB, "omm"),
                                        (psA, "mm")][ti % 3]
                            ops = pool.tile([128, SC_ATT], FP32, tag=tg)
                        else:
                            ops = psB.tile([128, SC_ATT], FP32, tag="omm")
                        ti += 1
                        for h in range(2):
                            nc.tensor.matmul(
                                ops, lhsT=attnT[h][:, bass.ts(st, 128)],
                                rhs=woTs[:, h, bass.ts(jc, SC_ATT)],
                                start=(h == 0), stop=(h == 1))
                        osb = opool.tile([128, SC_ATT], BF16, tag="osb",
                                         bufs=4)
                        # halve the PSUM-bank hold time: both engines copy
                        # one half of the tile in parallel
                        nc.vector.tensor_copy(osb[:, 0:256], ops[:, 0:256])
                        nc.scalar.copy(osb[:, 256:512], ops[:, 256:512])
                        nc.sync.dma_start(
                            out=outp[bass.ts(st, 128), bass.ts(jc, SC_ATT)],
                            in_=osb)

    nc.compile()
    return nc


def _prep_inputs(hidden_states, cos, sin, wqkv, bqkv, wo, q_norm_w, k_norm_w):
    """Host-side layout prep + per-core sharding. All device tensors are
    pre-swizzled (and converted to bf16) so every DMA has long contiguous
    per-partition runs at half the fp32 byte count."""
    import ml_dtypes
    f32 = np.float32
    bf16 = ml_dtypes.bfloat16
    hTn = np.ascontiguousarray(hidden_states.reshape(S, HID).T).astype(f32)
    hTh = np.ascontiguousarray(
        hTn.reshape(16, 128, S).transpose(1, 0, 2)).astype(bf16)  # [p, kt, s]
    cosT = cos.T.astype(f32)  # [64, S]
    sinT = sin.T.astype(f32)
    cs2 = np.ascontiguousarray(
        np.concatenate([cosT, cosT], axis=0)).astype(bf16)
    ss2 = np.ascontiguousarray(
        np.concatenate([sinT, sinT], axis=0)).astype(bf16)
    qw = np.ascontiguousarray(q_norm_w.reshape(128, 1)).astype(f32)
    qws = np.ascontiguousarray(
        np.concatenate([q_norm_w[64:], q_norm_w[:64]]).reshape(128, 1)).astype(f32)
    kw = np.ascontiguousarray(k_norm_w.reshape(128, 1)).astype(f32)
    kws = np.ascontiguousarray(
        np.concatenate([k_norm_w[64:], k_norm_w[:64]]).reshape(128, 1)).astype(f32)
    ones_np = np.ones((128, 128), dtype=f32)
    onesb_np = np.ones((128, 128), dtype=bf16)
    rt = np.zeros((128, 128), dtype=f32)
    rt[np.arange(64) + 64, np.arange(64)] = -1.0   # R^T[d+64, d] = -1
    rt[np.arange(64), np.arange(64) + 64] = 1.0    # R^T[d-64, d] = +1
    rt = rt.astype(bf16)
    # causal mask template: tmask[p, 512+d] = -1e9 where d < p
    dcol = np.arange(2 * 512)[None, :] - 512
    prow = np.arange(128)[:, None]
    tmask = np.where(dcol < prow, np.float32(-1e9), np.float32(0.0)).astype(bf16)

    in_maps = []
    for c in range(N_CORES):
        kvh = c // 2
        rows = list(range(2 * c * HD, (2 * c + 2) * HD))          # q0, q1
        rows += list(range(NH * HD + kvh * HD, NH * HD + (kvh + 1) * HD))  # k
        rows += list(range((NH + NKV) * HD + kvh * HD,
                           (NH + NKV) * HD + (kvh + 1) * HD))      # v
        w_c = wqkv[rows]                       # [512, HID]
        wTc = np.ascontiguousarray(w_c.T).astype(f32)   # [HID, 512]
        wTk = wTc.reshape(16, 128, 512)
        b_c = bqkv[rows].astype(f32)           # [512]
        b4c = np.ascontiguousarray(b_c.reshape(4, 128).T)  # [128, 4]
        woc = wo[:, 2 * c * HD:(2 * c + 2) * HD]  # [HID, 256]
        woTc = np.ascontiguousarray(woc.T).astype(f32)  # [256, HID]
        woTh = np.ascontiguousarray(
            woTc.reshape(2, 128, HID).transpose(1, 0, 2)).astype(bf16)
        im = {
            "hT": hTh, "b4": b4c, "woT": woTh,
            "cs2": cs2, "ss2": ss2,
            "qw": qw, "qws": qws, "kw": kw, "kws": kws,
            "ones": ones_np, "onesb": onesb_np, "rswap": rt,
            "tmask": tmask,
        }
        for oc in range(4):
            im[f"wT{oc}"] = np.ascontiguousarray(
                wTk[:, :, oc * 128:(oc + 1) * 128].transpose(1, 0, 2)).astype(
                    bf16)
        in_maps.append(im)
    return in_maps


_NC_CACHE = {}


def kernel(hidden_states, cos, sin, k_cache, v_cache, mask,
           wqkv, bqkv, wo, bo, q_norm_w, k_norm_w, kv_write_indices,
           trace=False):
    hidden_states = np.asarray(hidden_states, dtype=np.float32)
    in_maps = _prep_inputs(
        np.asarray(hidden_states), np.asarray(cos), np.asarray(sin),
        np.asarray(wqkv), np.asarray(bqkv), np.asarray(wo),
        np.asarray(q_norm_w), np.asarray(k_norm_w))

    if "nc" not in _NC_CACHE:
        _NC_CACHE["nc"] = build_nc()
    nc = _NC_CACHE["nc"]

    res = run_bass_kernel_spmd(nc, in_maps, core_ids=list(range(N_CORES)),
                               trace=trace)
    out = np.zeros((S, HID), np.float32)
    for rmap in res.results:
        out += np.asarray(rmap["outp"], dtype=np.float32)
    out += np.asarray(bo, dtype=np.float32)[None, :]
    if trace:
        kernel.last_results = res
    return out.reshape(1, S, HID)
